# revision 10
# baseline (speedup 1.0000x reference)
"""GIN-style 5-layer GNN message passing on 8 Trainium2 NeuronCores.

Strategy v6 (1D node-parallel, tensor-engine aggregation, 4 SWDGE queues):
  - Nodes partitioned contiguously across 8 cores (12500 each, padded to
    12544 = 98*128). Edges owned by their dst core. The bottleneck is
    GPSIMD/SWDGE per-edge gather-descriptor generation (~8 ns/idx per
    queue); descriptor generation parallelizes ~4x across the 4 SWDGE
    queues, so gather calls round-robin queues 0-3.
  - Per layer: two AllGathers (shard halves, bf16, ping-ponged DRAM
    buffers) build the full node table; dst tiles are processed
    half-major (A-half tiles fully finish, their AllGather for the next
    layer is emitted at the half boundary, then B-half) so collectives
    hide under the other half's gather work.
  - dma_gather fetches h[src] per edge (256B rows) in dst-tile-sorted
    order; cells are (tile, window) with CAP=512 slots; per-call stray
    segments (uniform size = max overflow over cores, rounded to 128)
    absorb cell overflow, so there is no scatter-add path at all.
  - Aggregation on the tensor engine: per 128-edge group a one-hot fp8
    selection matrix contracts gathered bf16 messages into PSUM
    (out[dst, feat] += sel^T @ msg). Self-loops ride the bank-filling
    first matmul (identity x h_own), edge embeddings fold into a counts
    matmul (counts^T @ etab). Stray groups get one sel matmul per tile
    of the block.
  - GIN MLP (D->2D->relu->D) + BatchNorm folded into the second linear,
    bf16 weights, per 128-node tile on the tensor engine.
"""
import sys
import numpy as np

sys.path.insert(0, "/opt/trn_rl_repo")

import ml_dtypes
import concourse.bass as bass
import concourse.bacc as bacc
import concourse.tile as tile
import concourse.masks as masks
from concourse import mybir
from concourse.bass_utils import run_bass_kernel_spmd


class CFG:
    NQ = 4              # SWDGE queues (desc-gen parallelism)
    N = 100000          # total nodes
    D = 128             # feature dim
    L = 5               # layers
    NCORE = 8
    NOWN = 12500        # nodes per core
    NPAD = 12544        # padded nodes per core (98 * 128)
    NBLK = 4            # gather source windows (int16 idx limit)
    TPB = 7             # dst tiles per block (49 = 7*7 per half)
    CAP = 512           # slot capacity per (tile, window) cell
    STRAY = 256         # stray slots per (block, window); set from data
    EPS = 1e-5

    @property
    def WIN(self):      # rows per gather window in h_full space
        return 2 * self.NPAD

    @property
    def NTILE(self):
        return self.NPAD // 128

    @property
    def GPC(self):      # main groups per cell
        return self.CAP // 128

    @property
    def SG(self):       # stray groups per call
        return self.STRAY // 128

    @property
    def HTILE(self):    # tiles per half (49)
        return self.NTILE // 2

    @property
    def halves(self):   # [[blocks of half A], [blocks of half B]]
        out = []
        for h in range(2):
            t0 = h * self.HTILE
            ts = list(range(t0, t0 + self.HTILE))
            out.append([ts[i:i + self.TPB]
                        for i in range(0, self.HTILE, self.TPB)])
        return out

    @property
    def calls(self):    # canonical emission order: (half, w, blk)
        out = []
        for h in range(2):
            for w in range(self.NBLK):
                for blk in self.halves[h]:
                    out.append((h, w, blk))
        return out

    def call_lidx(self, blk):   # idxs in one call
        return len(blk) * self.CAP + self.STRAY

    def call_nsel(self, blk):   # sel matrices in one call
        return len(blk) * self.GPC + self.SG * len(blk)

    @property
    def TOTIDX(self):
        return sum(self.call_lidx(blk) for _, _, blk in self.calls)

    @property
    def TOTSEL(self):
        return sum(self.call_nsel(blk) for _, _, blk in self.calls)


def _f8(a):
    return np.asarray(a, np.float32).astype(ml_dtypes.float8_e4m3fn)


def _bf(a):
    return np.asarray(a, np.float32).astype(ml_dtypes.bfloat16)


def _fold_params(cfg, x_emb, etab, w1, b1, w2, b2, gamma, beta, bn_mean, bn_var):
    """Host-side parameter folding. Returns replicated device param arrays."""
    D, L = cfg.D, cfg.L
    x_emb = np.asarray(x_emb, np.float64)
    etab = np.asarray(etab, np.float64)
    w1 = np.asarray(w1, np.float64)
    b1 = np.asarray(b1, np.float64)
    w2 = np.asarray(w2, np.float64)
    b2 = np.asarray(b2, np.float64)
    gamma = np.asarray(gamma, np.float64)
    beta = np.asarray(beta, np.float64)
    bn_mean = np.asarray(bn_mean, np.float64)
    bn_var = np.asarray(bn_var, np.float64)

    xemb6 = np.zeros((8, D), np.float64)
    xemb6[0:3] = x_emb[0:3]
    xemb6[3:6] = x_emb[120:123]

    etab9 = np.zeros((L, 16, D), np.float64)
    etab9[:, 0:9, :] = etab

    w1t = np.ascontiguousarray(np.transpose(w1, (0, 2, 1)))          # [L,D,2D]
    b1t = np.ascontiguousarray(
        b1.reshape(L, 2, D).transpose(0, 2, 1)).astype(np.float32)   # [L,D,2]

    s = gamma / np.sqrt(bn_var + cfg.EPS)          # [L, D]
    t = beta - bn_mean * s
    w2f = w2 * s[:, :, None]                       # [L, D, 2D] rows scaled
    b2f = b2 * s + t                               # [L, D]
    # stationary chunks: w2s[l, p, k, m] = w2f[l, m, k*128 + p]
    w2s = np.ascontiguousarray(
        np.transpose(w2f.reshape(L, D, 2, D), (0, 3, 2, 1)))         # [L,128,2,128]
    b2t = b2f.astype(np.float32).reshape(L, D, 1)
    return dict(xemb6=_bf(xemb6), etab9=_bf(etab9), w1t=_bf(w1t), b1t=b1t,
                w2s=_bf(w2s), b2t=b2t)


def _wrap16(a):
    """Element i -> [i % 16, i // 16], replicated to 128 partitions."""
    assert len(a) % 16 == 0
    w = a.reshape(-1, 16).T
    return np.ascontiguousarray(np.tile(w, (8, 1)))


def _edge_wtj(cfg, src_g, dst_l):
    """Per-edge (window, window-local idx, tile, col)."""
    HALF = cfg.NPAD // 2
    q = src_g // cfg.NOWN
    local = src_g - q * cfg.NOWN
    in_a = local < HALF
    halfrow = np.where(in_a, q * HALF + local, q * HALF + local - HALF)
    w = halfrow // cfg.WIN + np.where(in_a, 0, 2)
    widx = (halfrow % cfg.WIN).astype(np.int64)
    assert widx.max() < 2 ** 15
    t = dst_l // 128
    j = dst_l % 128
    return w, widx, t, j


def _core_cells(cfg, src_g, dst_l):
    """cells[(t, w)] -> (widx array, j array), sorted deterministically."""
    w, widx, t, j = _edge_wtj(cfg, src_g, dst_l)
    order = np.lexsort((widx, j, w, t))
    w, widx, t, j = w[order], widx[order], t[order], j[order]
    cells = {}
    key = t * cfg.NBLK + w
    uniq, starts = np.unique(key, return_index=True)
    bounds = list(starts) + [len(key)]
    for i, k in enumerate(uniq):
        s, e = bounds[i], bounds[i + 1]
        cells[(int(k) // cfg.NBLK, int(k) % cfg.NBLK)] = (widx[s:e], j[s:e])
    return cells


def _stray_need(cfg, cells):
    """Max stray slots needed over (block, window) for one core."""
    need = 0
    for _, w, blk in cfg.calls:
        tot = sum(max(len(cells.get((t, w), ((), ()))[0]) - cfg.CAP, 0)
                  for t in blk)
        need = max(need, tot)
    return need


def _schedule_core(cfg, cells):
    """Emit gidx [TOTIDX] int16 and selT [128, TOTSEL, 128] fp8."""
    gidx = np.zeros(cfg.TOTIDX, np.int16)
    selT = np.zeros((128, cfg.TOTSEL, 128), ml_dtypes.float8_e4m3fn)
    ioff = 0
    soff = 0
    for _, w, blk in cfg.calls:
        nt = len(blk)
        # main cells
        for ti, t in enumerate(blk):
            widx, j = cells.get((t, w), (np.zeros(0, np.int64),) * 2)
            n = min(len(widx), cfg.CAP)
            base = ioff + ti * cfg.CAP
            gidx[base:base + n] = widx[:n].astype(np.int16)
            slot = np.arange(n)
            selT[slot % 128, soff + ti * cfg.GPC + slot // 128, j[:n]] = 1.0
        # stray segment
        sbase = ioff + nt * cfg.CAP
        sgsel = soff + nt * cfg.GPC
        pos = 0
        for ti, t in enumerate(blk):
            widx, j = cells.get((t, w), (np.zeros(0, np.int64),) * 2)
            if len(widx) <= cfg.CAP:
                continue
            ow, oj = widx[cfg.CAP:], j[cfg.CAP:]
            n = len(ow)
            assert pos + n <= cfg.STRAY, "stray segment overflow"
            slot = pos + np.arange(n)
            gidx[sbase + slot] = ow.astype(np.int16)
            # stray group g, tile ti -> sel index sgsel + g*nt + ti
            selT[slot % 128, sgsel + (slot // 128) * nt + ti, oj] = 1.0
            pos += n
        ioff += cfg.call_lidx(blk)
        soff += cfg.call_nsel(blk)
    assert ioff == cfg.TOTIDX and soff == cfg.TOTSEL
    return gidx, selT


def _prepare_inputs(cfg, x, edge_index, edge_attr):
    """Per-core host prep. Returns per-core dict list; sets cfg.STRAY."""
    x = np.asarray(x)
    src = np.asarray(edge_index[0], np.int64)
    dst = np.asarray(edge_index[1], np.int64)
    eb = np.asarray(edge_attr[:, 0], np.int64)
    ed = np.asarray(edge_attr[:, 1], np.int64)

    owner = dst // cfg.NOWN
    all_cells = []
    need = 0
    for r in range(cfg.NCORE):
        m = owner == r
        cells = _core_cells(cfg, src[m], dst[m] - r * cfg.NOWN)
        all_cells.append(cells)
        need = max(need, _stray_need(cfg, cells))
    cfg.STRAY = max(128, -(-need // 128) * 128)

    per_core = []
    for r in range(cfg.NCORE):
        m = owner == r
        dst_l = dst[m] - r * cfg.NOWN
        gidx, selT = _schedule_core(cfg, all_cells[r])

        countsT = np.zeros((16, cfg.NPAD), np.float32)
        np.add.at(countsT, (eb[m], dst_l), 1.0)
        np.add.at(countsT, (6 + ed[m], dst_l), 1.0)
        loc = np.arange(cfg.NOWN)
        countsT[4, loc] += 1.0   # self-loop bond type 4
        countsT[6, loc] += 1.0   # self-loop direction 0

        xohT = np.zeros((8, cfg.NPAD), np.float32)
        xl = np.asarray(x[r * cfg.NOWN:(r + 1) * cfg.NOWN], np.int64)
        xohT[xl[:, 0], loc] = 1.0
        xohT[3 + xl[:, 1], loc] += 1.0

        per_core.append(dict(
            gidx=_wrap16(gidx), selT=selT,
            countsT=_bf(countsT), xohT=_bf(xohT),
        ))
    return per_core


def _build_program(cfg):
    nc = bacc.Bacc(None, target_bir_lowering=False, debug=True,
                   num_swdge_queues=cfg.NQ)
    f32, bf16, i16 = mybir.dt.float32, mybir.dt.bfloat16, mybir.dt.int16
    fp8 = mybir.dt.float8e4
    D, L = cfg.D, cfg.L
    GPC, SG = cfg.GPC, cfg.SG
    HALF = cfg.NPAD // 2

    # I/O
    gidx_in = nc.dram_tensor("gidx", [128, cfg.TOTIDX // 16], i16,
                             kind="ExternalInput")
    selT_in = nc.dram_tensor("selT", [128, cfg.TOTSEL, 128], fp8,
                             kind="ExternalInput")
    countsT_in = nc.dram_tensor("countsT", [16, cfg.NPAD], bf16,
                                kind="ExternalInput")
    xohT_in = nc.dram_tensor("xohT", [8, cfg.NPAD], bf16, kind="ExternalInput")
    xemb6_in = nc.dram_tensor("xemb6", [8, D], bf16, kind="ExternalInput")
    etab9_in = nc.dram_tensor("etab9", [L, 16, D], bf16, kind="ExternalInput")
    w1t_in = nc.dram_tensor("w1t", [L, D, 2 * D], bf16, kind="ExternalInput")
    b1t_in = nc.dram_tensor("b1t", [L, D, 2], f32, kind="ExternalInput")
    w2s_in = nc.dram_tensor("w2s", [L, 128, 2, 128], bf16, kind="ExternalInput")
    b2t_in = nc.dram_tensor("b2t", [L, D, 1], f32, kind="ExternalInput")
    out_ext = nc.dram_tensor("out", [cfg.NPAD, D], f32, kind="ExternalOutput")

    # internal DRAM; hfull halves ping-pong across layers
    hownA = nc.dram_tensor("hownA", [HALF, D], bf16)
    hownB = nc.dram_tensor("hownB", [HALF, D], bf16)
    hfullA = [nc.dram_tensor(f"hfullA{p}", [cfg.NCORE * HALF, D], bf16,
                             addr_space="Shared") for p in range(2)]
    hfullB = [nc.dram_tensor(f"hfullB{p}", [cfg.NCORE * HALF, D], bf16,
                             addr_space="Shared") for p in range(2)]

    def hown_rows(t):
        r0 = t * 128
        if r0 < HALF:
            return hownA[r0:r0 + 128, :]
        return hownB[r0 - HALF:r0 - HALF + 128, :]

    relu = mybir.ActivationFunctionType.Relu
    import itertools
    qrr = itertools.cycle(range(cfg.NQ))
    MAXG = cfg.TPB * GPC + SG          # gather groups per call
    MAXSEL = cfg.TPB * (GPC + SG)      # sel matrices per call

    # per-call gidx/sel base offsets in emission order
    ibases, sbases = [], []
    ioff = soff = 0
    for _, w, blk in cfg.calls:
        ibases.append(ioff)
        sbases.append(soff)
        ioff += cfg.call_lidx(blk)
        soff += cfg.call_nsel(blk)

    with tile.TileContext(nc) as tc:
        with (
            tc.tile_pool(name="const", bufs=1) as const_pool,
            tc.tile_pool(name="gather", bufs=4) as gather_pool,
            tc.tile_pool(name="sel", bufs=4) as sel_pool,
            tc.tile_pool(name="cnt", bufs=2) as cnt_pool,
            tc.tile_pool(name="mlp", bufs=3) as mlp_pool,
            tc.tile_pool(name="aggp", bufs=2, space="PSUM") as agg_pool,
            tc.tile_pool(name="psA", bufs=1, space="PSUM") as psA_pool,
            tc.tile_pool(name="psB", bufs=2, space="PSUM") as psB_pool,
        ):
            # ---- resident constants ----
            identf = const_pool.tile([128, 128], f32, tag="identf")
            masks.make_identity(nc, identf[:, :])
            identb = const_pool.tile([128, 128], bf16, tag="identb")
            nc.vector.tensor_copy(identb[:, :], identf[:, :])
            zerob = const_pool.tile([128, 4, 128], bf16, tag="zerob")
            nc.gpsimd.memset(zerob[:, :, :], 0.0)
            agg_sb = const_pool.tile([128, cfg.NTILE, 128], f32, tag="agg_sb")
            gidx_t = const_pool.tile([128, cfg.TOTIDX // 16], i16, tag="gidx")
            nc.sync.dma_start(gidx_t[:, :], gidx_in[:, :])
            hown_sb = const_pool.tile([128, cfg.NTILE, 128], bf16, tag="hown_sb")
            xemb6 = const_pool.tile([8, D], bf16, tag="xemb6")
            nc.sync.dma_start(xemb6[:, :], xemb6_in[:, :])
            etab9 = [const_pool.tile([16, D], bf16, tag=f"etab9_{l}",
                                     name=f"etab9_{l}") for l in range(L)]
            w1t = [const_pool.tile([D, 2 * D], bf16, tag=f"w1t_{l}",
                                   name=f"w1t_{l}") for l in range(L)]
            b1t = [const_pool.tile([D, 2], f32, tag=f"b1t_{l}",
                                   name=f"b1t_{l}") for l in range(L)]
            w2s = [const_pool.tile([128, 2, 128], bf16, tag=f"w2s_{l}",
                                   name=f"w2s_{l}") for l in range(L)]
            b2t = [const_pool.tile([D, 1], f32, tag=f"b2t_{l}",
                                   name=f"b2t_{l}") for l in range(L)]
            for l in range(L):
                nc.sync.dma_start(etab9[l][:, :], etab9_in[l])
                nc.sync.dma_start(w1t[l][:, :], w1t_in[l])
                nc.sync.dma_start(b1t[l][:, :], b1t_in[l])
                nc.sync.dma_start(w2s[l][:, :, :], w2s_in[l])
                nc.sync.dma_start(b2t[l][:, :], b2t_in[l])

            def emit_ag(half, p):
                src_t = hownA if half == 0 else hownB
                dst_t = (hfullA if half == 0 else hfullB)[p]
                nc.gpsimd.collective_compute(
                    "AllGather", mybir.AluOpType.bypass,
                    ins=[src_t[:, :]], outs=[dst_t[:, :]],
                    replica_groups=[list(range(cfg.NCORE))],
                )

            # ---- layer-0 node embedding: h0 = onehot @ xemb6 ----
            AG1_TILE = cfg.HTILE - 1
            for t in range(cfg.NTILE):
                cols = slice(t * 128, (t + 1) * 128)
                xoh_t = mlp_pool.tile([8, 128], bf16, tag="xoh_t")
                nc.sync.dma_start(xoh_t[:, :], xohT_in[:, cols])
                h0p = psA_pool.tile([128, D], f32, tag="ps1")
                nc.tensor.matmul(h0p[:, :], xoh_t[:, :], xemb6[:, :],
                                 start=True, stop=True)
                nc.vector.tensor_copy(hown_sb[:, t, :], h0p[:, :])
                nc.sync.dma_start(hown_rows(t), hown_sb[:, t, :])
                if t == AG1_TILE:
                    emit_ag(0, 0)
            emit_ag(1, 0)

            # ---- layers ----
            def mlp_tile(l, t):
                tp = psA_pool.tile([128, D], f32, tag="ps1")
                nc.tensor.transpose(tp[:, :], agg_sb[:, t, :], identf[:, :])
                tS = mlp_pool.tile([128, D], bf16, tag="tS")
                nc.vector.tensor_copy(tS[:, :], tp[:, :])
                hm = psB_pool.tile([128, 2, 128], f32, tag="hm")
                hmS = mlp_pool.tile([128, 2, 128], bf16, tag="hmS")
                for jj in range(2):
                    nc.tensor.matmul(
                        hm[:, jj, :], w1t[l][:, jj * 128:(jj + 1) * 128],
                        tS[:, :], start=True, stop=True)
                    nc.scalar.activation(
                        hmS[:, jj, :], hm[:, jj, :], relu,
                        bias=b1t[l][:, jj:jj + 1])
                h2p = psA_pool.tile([128, D], f32, tag="ps1")
                for jj in range(2):
                    nc.tensor.matmul(
                        h2p[:, :], w2s[l][:, jj, :], hmS[:, jj, :],
                        start=(jj == 0), stop=(jj == 1))
                if l < L - 1:
                    h2S = mlp_pool.tile([128, D], bf16, tag="h2S")
                    nc.scalar.activation(
                        h2S[:, :], h2p[:, :], relu, bias=b2t[l][:, 0:1])
                    op = psA_pool.tile([128, D], bf16, tag="ps2")
                    nc.tensor.transpose(op[:, :], h2S[:, :], identb[:, :])
                    nc.vector.tensor_copy(hown_sb[:, t, :], op[:, :])
                    nc.sync.dma_start(hown_rows(t), hown_sb[:, t, :])
                else:
                    h2S = mlp_pool.tile([128, D], f32, tag="h2Sf")
                    nc.vector.tensor_scalar_add(
                        h2S[:, :], h2p[:, :], b2t[l][:, 0:1])
                    op = psA_pool.tile([128, D], f32, tag="ps2")
                    nc.tensor.transpose(op[:, :], h2S[:, :], identf[:, :])
                    oS = mlp_pool.tile([128, D], f32, tag="oSf")
                    nc.vector.tensor_copy(oS[:, :], op[:, :])
                    nc.sync.dma_start(out_ext[t * 128:(t + 1) * 128, :],
                                      oS[:, :])

            for l in range(L):
                p = l % 2
                ci = 0
                for half in range(2):
                    for w in range(cfg.NBLK):
                        src = (hfullA if w < 2 else hfullB)[p]
                        woff = (w % 2) * cfg.WIN
                        for blk in cfg.halves[half]:
                            nt = len(blk)
                            lidx = cfg.call_lidx(blk)
                            nsel = cfg.call_nsel(blk)
                            ngrp = lidx // 128
                            ib, sb = ibases[ci], sbases[ci]
                            ci += 1
                            gbuf = gather_pool.tile([128, MAXG, D], bf16,
                                                    tag="g")
                            nc.gpsimd.dma_gather(
                                gbuf[:, 0:ngrp, :],
                                src[woff:woff + cfg.WIN, :],
                                gidx_t[:, ib // 16:(ib + lidx) // 16],
                                lidx, lidx, D,
                                single_packet=False, queue_num=next(qrr))
                            sel_t = sel_pool.tile([128, MAXSEL, 128], fp8,
                                                  tag="sel")
                            nc.sync.dma_start(
                                sel_t[:, 0:nsel, :],
                                selT_in[:, sb:sb + nsel, :])
                            agg = agg_pool.tile([128, cfg.TPB, 128], f32,
                                                tag="agg")
                            # bank-filling first matmuls (<=4 tiles each)
                            for c0 in range(0, nt, 4):
                                c1 = min(c0 + 4, nt)
                                if w == 0:
                                    nc.tensor.matmul(
                                        agg[:, c0:c1, :], identb[:, :],
                                        hown_sb[:, blk[0] + c0:blk[0] + c1, :],
                                        start=True, stop=False,
                                        skip_group_check=True)
                                else:
                                    nc.tensor.matmul(
                                        agg[:, c0:c1, :], identb[:, :],
                                        zerob[:, 0:c1 - c0, :],
                                        start=True, stop=False,
                                        skip_group_check=True)
                            if w == 0:
                                cnt_t = cnt_pool.tile([16, cfg.TPB, 128],
                                                      bf16, tag="cnt")
                                nc.sync.dma_start(
                                    cnt_t[:, 0:nt, :],
                                    countsT_in[:, blk[0] * 128:
                                               (blk[0] + nt) * 128].rearrange(
                                        "p (a b) -> p a b", b=128))
                                for i in range(nt):
                                    nc.tensor.matmul(
                                        agg[:, i, :], cnt_t[:, i, :],
                                        etab9[l][:, :],
                                        start=False, stop=False,
                                        skip_group_check=True)
                            nmm = nt * GPC + SG * nt
                            mi = 0
                            for g in range(nt * GPC):
                                mi += 1
                                nc.tensor.matmul(
                                    agg[:, g // GPC, :], sel_t[:, g, :],
                                    gbuf[:, g, :],
                                    start=False, stop=(mi == nmm),
                                    skip_group_check=True)
                            for sg in range(SG):
                                for i in range(nt):
                                    mi += 1
                                    nc.tensor.matmul(
                                        agg[:, i, :],
                                        sel_t[:, nt * GPC + sg * nt + i, :],
                                        gbuf[:, nt * GPC + sg, :],
                                        start=False, stop=(mi == nmm),
                                        skip_group_check=True)
                            cols = slice(blk[0], blk[0] + nt)
                            if w == 0:
                                nc.vector.tensor_copy(agg_sb[:, cols, :],
                                                      agg[:, 0:nt, :])
                            else:
                                nc.vector.tensor_add(agg_sb[:, cols, :],
                                                     agg_sb[:, cols, :],
                                                     agg[:, 0:nt, :])
                            if w == cfg.NBLK - 1:
                                for t in blk:
                                    mlp_tile(l, t)
                    if l < L - 1:
                        emit_ag(half, (l + 1) % 2)

    nc.finalize()
    return nc


_CACHE = {}


def _get_program(cfg):
    key = (cfg.N, cfg.CAP, cfg.TPB, cfg.STRAY, cfg.NQ)
    if key not in _CACHE:
        _CACHE[key] = _build_program(cfg)
    return _CACHE[key]


def build_in_maps(cfg, inputs):
    params = _fold_params(
        cfg, inputs["x_emb"], inputs["etab"], inputs["w1"], inputs["b1"],
        inputs["w2"], inputs["b2"], inputs["gamma"], inputs["beta"],
        inputs["bn_mean"], inputs["bn_var"])
    per_core = _prepare_inputs(cfg, inputs["x"], inputs["edge_index"],
                               inputs["edge_attr"])
    in_maps = []
    for r in range(cfg.NCORE):
        m = dict(per_core[r])
        m.update({k: np.ascontiguousarray(v) for k, v in params.items()})
        in_maps.append(m)
    return in_maps


def kernel(**inputs) -> np.ndarray:
    cfg = CFG()
    in_maps = build_in_maps(cfg, inputs)   # sets cfg.STRAY from data
    nc = _get_program(cfg)
    res = run_bass_kernel_spmd(nc, in_maps, list(range(cfg.NCORE)))
    out = np.empty((cfg.N, cfg.D), np.float32)
    for r in range(cfg.NCORE):
        out[r * cfg.NOWN:(r + 1) * cfg.NOWN] = res.results[r]["out"][:cfg.NOWN]
    return out


# revision 11
# speedup vs baseline: 1.0905x; 1.0905x over previous
"""GIN-style 5-layer GNN message passing on 8 Trainium2 NeuronCores.

Strategy v6 (1D node-parallel, tensor-engine aggregation, 4 SWDGE queues):
  - Nodes partitioned contiguously across 8 cores (12500 each, padded to
    12544 = 98*128). Edges owned by their dst core. The bottleneck is
    GPSIMD/SWDGE per-edge gather-descriptor generation (~8 ns/idx per
    queue); descriptor generation parallelizes ~4x across the 4 SWDGE
    queues, so gather calls round-robin queues 0-3.
  - Per layer: two AllGathers (shard halves, bf16, ping-ponged DRAM
    buffers) build the full node table; dst tiles are processed
    half-major (A-half tiles fully finish, their AllGather for the next
    layer is emitted at the half boundary, then B-half) so collectives
    hide under the other half's gather work.
  - dma_gather fetches h[src] per edge (256B rows) in dst-tile-sorted
    order; cells are (tile, window) with CAP=512 slots; per-call stray
    segments (uniform size = max overflow over cores, rounded to 128)
    absorb cell overflow, so there is no scatter-add path at all.
  - Aggregation on the tensor engine: per 128-edge group a one-hot fp8
    selection matrix contracts gathered bf16 messages into PSUM
    (out[dst, feat] += sel^T @ msg). Self-loops ride the bank-filling
    first matmul (identity x h_own), edge embeddings fold into a counts
    matmul (counts^T @ etab). Stray groups get one sel matmul per tile
    of the block.
  - GIN MLP (D->2D->relu->D) + BatchNorm folded into the second linear,
    bf16 weights, per 128-node tile on the tensor engine.
"""
import sys
import numpy as np

sys.path.insert(0, "/opt/trn_rl_repo")

import ml_dtypes
import concourse.bass as bass
import concourse.bacc as bacc
import concourse.tile as tile
import concourse.masks as masks
from concourse import mybir
from concourse.bass_utils import run_bass_kernel_spmd


class CFG:
    NQ = 4              # SWDGE queues (desc-gen parallelism)
    N = 100000          # total nodes
    D = 128             # feature dim
    L = 5               # layers
    NCORE = 8
    NOWN = 12500        # nodes per core
    NPAD = 12544        # padded nodes per core (98 * 128)
    NBLK = 4            # gather source windows (int16 idx limit)
    TPB = 7             # dst tiles per block (49 = 7*7 per half)
    CAP = 512           # slot capacity per (tile, window) cell
    STRAY = 256         # stray slots per (block, window); set from data
    EPS = 1e-5

    @property
    def WIN(self):      # rows per gather window in h_full space
        return 2 * self.NPAD

    @property
    def NTILE(self):
        return self.NPAD // 128

    @property
    def GPC(self):      # main groups per cell
        return self.CAP // 128

    @property
    def SG(self):       # stray groups per call
        return self.STRAY // 128

    @property
    def HTILE(self):    # tiles per half (49)
        return self.NTILE // 2

    @property
    def halves(self):   # [[blocks of half A], [blocks of half B]]
        out = []
        for h in range(2):
            t0 = h * self.HTILE
            ts = list(range(t0, t0 + self.HTILE))
            out.append([ts[i:i + self.TPB]
                        for i in range(0, self.HTILE, self.TPB)])
        return out

    @property
    def calls(self):    # canonical emission order: (half, w, blk)
        out = []
        for h in range(2):
            for w in range(self.NBLK):
                for blk in self.halves[h]:
                    out.append((h, w, blk))
        return out

    def call_lidx(self, blk):   # idxs in one call
        return len(blk) * self.CAP + self.STRAY

    def call_nsel(self, blk):   # sel matrices in one call
        return len(blk) * self.GPC + self.SG * len(blk)

    @property
    def TOTIDX(self):
        return sum(self.call_lidx(blk) for _, _, blk in self.calls)

    @property
    def TOTSEL(self):
        return sum(self.call_nsel(blk) for _, _, blk in self.calls)


def _f8(a):
    return np.asarray(a, np.float32).astype(ml_dtypes.float8_e4m3fn)


def _bf(a):
    return np.asarray(a, np.float32).astype(ml_dtypes.bfloat16)


def _fold_params(cfg, x_emb, etab, w1, b1, w2, b2, gamma, beta, bn_mean, bn_var):
    """Host-side parameter folding. Returns replicated device param arrays."""
    D, L = cfg.D, cfg.L
    x_emb = np.asarray(x_emb, np.float64)
    etab = np.asarray(etab, np.float64)
    w1 = np.asarray(w1, np.float64)
    b1 = np.asarray(b1, np.float64)
    w2 = np.asarray(w2, np.float64)
    b2 = np.asarray(b2, np.float64)
    gamma = np.asarray(gamma, np.float64)
    beta = np.asarray(beta, np.float64)
    bn_mean = np.asarray(bn_mean, np.float64)
    bn_var = np.asarray(bn_var, np.float64)

    xemb6 = np.zeros((8, D), np.float64)
    xemb6[0:3] = x_emb[0:3]
    xemb6[3:6] = x_emb[120:123]

    etab9 = np.zeros((L, 16, D), np.float64)
    etab9[:, 0:9, :] = etab

    w1t = np.ascontiguousarray(np.transpose(w1, (0, 2, 1)))          # [L,D,2D]
    b1t = np.ascontiguousarray(
        b1.reshape(L, 2, D).transpose(0, 2, 1)).astype(np.float32)   # [L,D,2]

    s = gamma / np.sqrt(bn_var + cfg.EPS)          # [L, D]
    t = beta - bn_mean * s
    w2f = w2 * s[:, :, None]                       # [L, D, 2D] rows scaled
    b2f = b2 * s + t                               # [L, D]
    # stationary chunks: w2s[l, p, k, m] = w2f[l, m, k*128 + p]
    w2s = np.ascontiguousarray(
        np.transpose(w2f.reshape(L, D, 2, D), (0, 3, 2, 1)))         # [L,128,2,128]
    b2t = b2f.astype(np.float32).reshape(L, D, 1)
    return dict(xemb6=_bf(xemb6), etab9=_bf(etab9), w1t=_bf(w1t), b1t=b1t,
                w2s=_bf(w2s), b2t=b2t)


def _wrap16(a):
    """Element i -> [i % 16, i // 16], replicated to 128 partitions."""
    assert len(a) % 16 == 0
    w = a.reshape(-1, 16).T
    return np.ascontiguousarray(np.tile(w, (8, 1)))


def _edge_wtj(cfg, src_g, dst_l):
    """Per-edge (window, window-local idx, tile, col)."""
    HALF = cfg.NPAD // 2
    q = src_g // cfg.NOWN
    local = src_g - q * cfg.NOWN
    in_a = local < HALF
    halfrow = np.where(in_a, q * HALF + local, q * HALF + local - HALF)
    w = halfrow // cfg.WIN + np.where(in_a, 0, 2)
    widx = (halfrow % cfg.WIN).astype(np.int64)
    assert widx.max() < 2 ** 15
    t = dst_l // 128
    j = dst_l % 128
    return w, widx, t, j


def _core_cells(cfg, src_g, dst_l):
    """cells[(t, w)] -> (widx array, j array), sorted deterministically."""
    w, widx, t, j = _edge_wtj(cfg, src_g, dst_l)
    order = np.lexsort((widx, j, w, t))
    w, widx, t, j = w[order], widx[order], t[order], j[order]
    cells = {}
    key = t * cfg.NBLK + w
    uniq, starts = np.unique(key, return_index=True)
    bounds = list(starts) + [len(key)]
    for i, k in enumerate(uniq):
        s, e = bounds[i], bounds[i + 1]
        cells[(int(k) // cfg.NBLK, int(k) % cfg.NBLK)] = (widx[s:e], j[s:e])
    return cells


def _stray_need(cfg, cells):
    """Max stray slots needed over (block, window) for one core."""
    need = 0
    for _, w, blk in cfg.calls:
        tot = sum(max(len(cells.get((t, w), ((), ()))[0]) - cfg.CAP, 0)
                  for t in blk)
        need = max(need, tot)
    return need


def _schedule_core(cfg, cells):
    """Emit gidx [TOTIDX] int16 and selT [128, TOTSEL, 128] fp8."""
    gidx = np.zeros(cfg.TOTIDX, np.int16)
    selT = np.zeros((128, cfg.TOTSEL, 128), ml_dtypes.float8_e4m3fn)
    ioff = 0
    soff = 0
    for _, w, blk in cfg.calls:
        nt = len(blk)
        # main cells
        for ti, t in enumerate(blk):
            widx, j = cells.get((t, w), (np.zeros(0, np.int64),) * 2)
            n = min(len(widx), cfg.CAP)
            base = ioff + ti * cfg.CAP
            gidx[base:base + n] = widx[:n].astype(np.int16)
            slot = np.arange(n)
            selT[slot % 128, soff + ti * cfg.GPC + slot // 128, j[:n]] = 1.0
        # stray segment
        sbase = ioff + nt * cfg.CAP
        sgsel = soff + nt * cfg.GPC
        pos = 0
        for ti, t in enumerate(blk):
            widx, j = cells.get((t, w), (np.zeros(0, np.int64),) * 2)
            if len(widx) <= cfg.CAP:
                continue
            ow, oj = widx[cfg.CAP:], j[cfg.CAP:]
            n = len(ow)
            assert pos + n <= cfg.STRAY, "stray segment overflow"
            slot = pos + np.arange(n)
            gidx[sbase + slot] = ow.astype(np.int16)
            # stray group g, tile ti -> sel index sgsel + g*nt + ti
            selT[slot % 128, sgsel + (slot // 128) * nt + ti, oj] = 1.0
            pos += n
        ioff += cfg.call_lidx(blk)
        soff += cfg.call_nsel(blk)
    assert ioff == cfg.TOTIDX and soff == cfg.TOTSEL
    return gidx, selT


def _prepare_inputs(cfg, x, edge_index, edge_attr):
    """Per-core host prep. Returns per-core dict list; sets cfg.STRAY."""
    x = np.asarray(x)
    src = np.asarray(edge_index[0], np.int64)
    dst = np.asarray(edge_index[1], np.int64)
    eb = np.asarray(edge_attr[:, 0], np.int64)
    ed = np.asarray(edge_attr[:, 1], np.int64)

    owner = dst // cfg.NOWN
    all_cells = []
    need = 0
    for r in range(cfg.NCORE):
        m = owner == r
        cells = _core_cells(cfg, src[m], dst[m] - r * cfg.NOWN)
        all_cells.append(cells)
        need = max(need, _stray_need(cfg, cells))
    cfg.STRAY = max(128, -(-need // 128) * 128)

    per_core = []
    for r in range(cfg.NCORE):
        m = owner == r
        dst_l = dst[m] - r * cfg.NOWN
        gidx, selT = _schedule_core(cfg, all_cells[r])

        countsT = np.zeros((16, cfg.NPAD), np.float32)
        np.add.at(countsT, (eb[m], dst_l), 1.0)
        np.add.at(countsT, (6 + ed[m], dst_l), 1.0)
        loc = np.arange(cfg.NOWN)
        countsT[4, loc] += 1.0   # self-loop bond type 4
        countsT[6, loc] += 1.0   # self-loop direction 0

        xohT = np.zeros((8, cfg.NPAD), np.float32)
        xl = np.asarray(x[r * cfg.NOWN:(r + 1) * cfg.NOWN], np.int64)
        xohT[xl[:, 0], loc] = 1.0
        xohT[3 + xl[:, 1], loc] += 1.0

        per_core.append(dict(
            gidx=_wrap16(gidx), selT=selT,
            countsT=_bf(countsT), xohT=_bf(xohT),
        ))
    return per_core


def _build_program(cfg):
    nc = bacc.Bacc(None, target_bir_lowering=False, debug=True,
                   num_swdge_queues=cfg.NQ)
    f32, bf16, i16 = mybir.dt.float32, mybir.dt.bfloat16, mybir.dt.int16
    fp8 = mybir.dt.float8e4
    D, L = cfg.D, cfg.L
    GPC, SG = cfg.GPC, cfg.SG
    HALF = cfg.NPAD // 2

    # I/O
    gidx_in = nc.dram_tensor("gidx", [128, cfg.TOTIDX // 16], i16,
                             kind="ExternalInput")
    selT_in = nc.dram_tensor("selT", [128, cfg.TOTSEL, 128], fp8,
                             kind="ExternalInput")
    countsT_in = nc.dram_tensor("countsT", [16, cfg.NPAD], bf16,
                                kind="ExternalInput")
    xohT_in = nc.dram_tensor("xohT", [8, cfg.NPAD], bf16, kind="ExternalInput")
    xemb6_in = nc.dram_tensor("xemb6", [8, D], bf16, kind="ExternalInput")
    etab9_in = nc.dram_tensor("etab9", [L, 16, D], bf16, kind="ExternalInput")
    w1t_in = nc.dram_tensor("w1t", [L, D, 2 * D], bf16, kind="ExternalInput")
    b1t_in = nc.dram_tensor("b1t", [L, D, 2], f32, kind="ExternalInput")
    w2s_in = nc.dram_tensor("w2s", [L, 128, 2, 128], bf16, kind="ExternalInput")
    b2t_in = nc.dram_tensor("b2t", [L, D, 1], f32, kind="ExternalInput")
    out_ext = nc.dram_tensor("out", [cfg.NPAD, D], f32, kind="ExternalOutput")

    # internal DRAM; hfull halves ping-pong across layers
    hownA = nc.dram_tensor("hownA", [HALF, D], bf16)
    hownB = nc.dram_tensor("hownB", [HALF, D], bf16)
    hfullA = [nc.dram_tensor(f"hfullA{p}", [cfg.NCORE * HALF, D], bf16,
                             addr_space="Shared") for p in range(2)]
    hfullB = [nc.dram_tensor(f"hfullB{p}", [cfg.NCORE * HALF, D], bf16,
                             addr_space="Shared") for p in range(2)]

    def hown_rows(t):
        r0 = t * 128
        if r0 < HALF:
            return hownA[r0:r0 + 128, :]
        return hownB[r0 - HALF:r0 - HALF + 128, :]

    relu = mybir.ActivationFunctionType.Relu
    import itertools
    qrr = itertools.cycle(range(cfg.NQ))
    MAXG = cfg.TPB * GPC + SG          # gather groups per call
    MAXSEL = cfg.TPB * (GPC + SG)      # sel matrices per call

    # per-call gidx/sel base offsets in emission order
    ibases, sbases = [], []
    ioff = soff = 0
    for _, w, blk in cfg.calls:
        ibases.append(ioff)
        sbases.append(soff)
        ioff += cfg.call_lidx(blk)
        soff += cfg.call_nsel(blk)

    with tile.TileContext(nc) as tc:
        with (
            tc.tile_pool(name="const", bufs=1) as const_pool,
            tc.tile_pool(name="gather", bufs=8) as gather_pool,
            tc.tile_pool(name="sel", bufs=5) as sel_pool,
            tc.tile_pool(name="cnt", bufs=2) as cnt_pool,
            tc.tile_pool(name="mlp", bufs=3) as mlp_pool,
            tc.tile_pool(name="aggp", bufs=2, space="PSUM") as agg_pool,
            tc.tile_pool(name="psA", bufs=1, space="PSUM") as psA_pool,
            tc.tile_pool(name="psB", bufs=2, space="PSUM") as psB_pool,
        ):
            # ---- resident constants ----
            identf = const_pool.tile([128, 128], f32, tag="identf")
            masks.make_identity(nc, identf[:, :])
            identb = const_pool.tile([128, 128], bf16, tag="identb")
            nc.vector.tensor_copy(identb[:, :], identf[:, :])
            zerob = const_pool.tile([128, 4, 128], bf16, tag="zerob")
            nc.gpsimd.memset(zerob[:, :, :], 0.0)
            agg_sb = const_pool.tile([128, cfg.NTILE, 128], f32, tag="agg_sb")
            gidx_t = const_pool.tile([128, cfg.TOTIDX // 16], i16, tag="gidx")
            nc.sync.dma_start(gidx_t[:, :], gidx_in[:, :])
            hown_sb = const_pool.tile([128, cfg.NTILE, 128], bf16, tag="hown_sb")
            xemb6 = const_pool.tile([8, D], bf16, tag="xemb6")
            nc.sync.dma_start(xemb6[:, :], xemb6_in[:, :])
            etab9 = [const_pool.tile([16, D], bf16, tag=f"etab9_{l}",
                                     name=f"etab9_{l}") for l in range(L)]
            w1t = [const_pool.tile([D, 2 * D], bf16, tag=f"w1t_{l}",
                                   name=f"w1t_{l}") for l in range(L)]
            b1t = [const_pool.tile([D, 2], f32, tag=f"b1t_{l}",
                                   name=f"b1t_{l}") for l in range(L)]
            w2s = [const_pool.tile([128, 2, 128], bf16, tag=f"w2s_{l}",
                                   name=f"w2s_{l}") for l in range(L)]
            b2t = [const_pool.tile([D, 1], f32, tag=f"b2t_{l}",
                                   name=f"b2t_{l}") for l in range(L)]
            for l in range(L):
                nc.sync.dma_start(etab9[l][:, :], etab9_in[l])
                nc.sync.dma_start(w1t[l][:, :], w1t_in[l])
                nc.sync.dma_start(b1t[l][:, :], b1t_in[l])
                nc.sync.dma_start(w2s[l][:, :, :], w2s_in[l])
                nc.sync.dma_start(b2t[l][:, :], b2t_in[l])

            def emit_ag(half, p):
                src_t = hownA if half == 0 else hownB
                dst_t = (hfullA if half == 0 else hfullB)[p]
                nc.gpsimd.collective_compute(
                    "AllGather", mybir.AluOpType.bypass,
                    ins=[src_t[:, :]], outs=[dst_t[:, :]],
                    replica_groups=[list(range(cfg.NCORE))],
                )

            # ---- layer-0 node embedding: h0 = onehot @ xemb6 ----
            AG1_TILE = cfg.HTILE - 1
            for t in range(cfg.NTILE):
                cols = slice(t * 128, (t + 1) * 128)
                xoh_t = mlp_pool.tile([8, 128], bf16, tag="xoh_t")
                nc.sync.dma_start(xoh_t[:, :], xohT_in[:, cols])
                h0p = psA_pool.tile([128, D], f32, tag="ps1")
                nc.tensor.matmul(h0p[:, :], xoh_t[:, :], xemb6[:, :],
                                 start=True, stop=True)
                nc.vector.tensor_copy(hown_sb[:, t, :], h0p[:, :])
                nc.sync.dma_start(hown_rows(t), hown_sb[:, t, :])
                if t == AG1_TILE:
                    emit_ag(0, 0)
            emit_ag(1, 0)

            # ---- layers ----
            def mlp_tile(l, t):
                tp = psA_pool.tile([128, D], f32, tag="ps1")
                nc.tensor.transpose(tp[:, :], agg_sb[:, t, :], identf[:, :])
                tS = mlp_pool.tile([128, D], bf16, tag="tS")
                nc.vector.tensor_copy(tS[:, :], tp[:, :])
                hm = psB_pool.tile([128, 2, 128], f32, tag="hm")
                hmS = mlp_pool.tile([128, 2, 128], bf16, tag="hmS")
                for jj in range(2):
                    nc.tensor.matmul(
                        hm[:, jj, :], w1t[l][:, jj * 128:(jj + 1) * 128],
                        tS[:, :], start=True, stop=True)
                    nc.scalar.activation(
                        hmS[:, jj, :], hm[:, jj, :], relu,
                        bias=b1t[l][:, jj:jj + 1])
                h2p = psA_pool.tile([128, D], f32, tag="ps1")
                for jj in range(2):
                    nc.tensor.matmul(
                        h2p[:, :], w2s[l][:, jj, :], hmS[:, jj, :],
                        start=(jj == 0), stop=(jj == 1))
                if l < L - 1:
                    h2S = mlp_pool.tile([128, D], bf16, tag="h2S")
                    nc.scalar.activation(
                        h2S[:, :], h2p[:, :], relu, bias=b2t[l][:, 0:1])
                    op = psA_pool.tile([128, D], bf16, tag="ps2")
                    nc.tensor.transpose(op[:, :], h2S[:, :], identb[:, :])
                    nc.vector.tensor_copy(hown_sb[:, t, :], op[:, :])
                    nc.sync.dma_start(hown_rows(t), hown_sb[:, t, :])
                else:
                    h2S = mlp_pool.tile([128, D], f32, tag="h2Sf")
                    nc.vector.tensor_scalar_add(
                        h2S[:, :], h2p[:, :], b2t[l][:, 0:1])
                    op = psA_pool.tile([128, D], f32, tag="ps2")
                    nc.tensor.transpose(op[:, :], h2S[:, :], identf[:, :])
                    oS = mlp_pool.tile([128, D], f32, tag="oSf")
                    nc.vector.tensor_copy(oS[:, :], op[:, :])
                    nc.sync.dma_start(out_ext[t * 128:(t + 1) * 128, :],
                                      oS[:, :])

            for l in range(L):
                p = l % 2
                ci = 0
                for half in range(2):
                    for w in range(cfg.NBLK):
                        src = (hfullA if w < 2 else hfullB)[p]
                        woff = (w % 2) * cfg.WIN
                        for blk in cfg.halves[half]:
                            nt = len(blk)
                            lidx = cfg.call_lidx(blk)
                            nsel = cfg.call_nsel(blk)
                            ngrp = lidx // 128
                            ib, sb = ibases[ci], sbases[ci]
                            ci += 1
                            gbuf = gather_pool.tile([128, MAXG, D], bf16,
                                                    tag="g")
                            nc.gpsimd.dma_gather(
                                gbuf[:, 0:ngrp, :],
                                src[woff:woff + cfg.WIN, :],
                                gidx_t[:, ib // 16:(ib + lidx) // 16],
                                lidx, lidx, D,
                                single_packet=False, queue_num=next(qrr))
                            sel_t = sel_pool.tile([128, MAXSEL, 128], fp8,
                                                  tag="sel")
                            nc.sync.dma_start(
                                sel_t[:, 0:nsel, :],
                                selT_in[:, sb:sb + nsel, :])
                            agg = agg_pool.tile([128, cfg.TPB, 128], f32,
                                                tag="agg")
                            # bank-filling first matmuls (<=4 tiles each)
                            for c0 in range(0, nt, 4):
                                c1 = min(c0 + 4, nt)
                                if w == 0:
                                    nc.tensor.matmul(
                                        agg[:, c0:c1, :], identb[:, :],
                                        hown_sb[:, blk[0] + c0:blk[0] + c1, :],
                                        start=True, stop=False,
                                        skip_group_check=True)
                                else:
                                    nc.tensor.matmul(
                                        agg[:, c0:c1, :], identb[:, :],
                                        zerob[:, 0:c1 - c0, :],
                                        start=True, stop=False,
                                        skip_group_check=True)
                            if w == 0:
                                cnt_t = cnt_pool.tile([16, cfg.TPB, 128],
                                                      bf16, tag="cnt")
                                nc.sync.dma_start(
                                    cnt_t[:, 0:nt, :],
                                    countsT_in[:, blk[0] * 128:
                                               (blk[0] + nt) * 128].rearrange(
                                        "p (a b) -> p a b", b=128))
                                for i in range(nt):
                                    nc.tensor.matmul(
                                        agg[:, i, :], cnt_t[:, i, :],
                                        etab9[l][:, :],
                                        start=False, stop=False,
                                        skip_group_check=True)
                            nmm = nt * GPC + SG * nt
                            mi = 0
                            for g in range(nt * GPC):
                                mi += 1
                                nc.tensor.matmul(
                                    agg[:, g // GPC, :], sel_t[:, g, :],
                                    gbuf[:, g, :],
                                    start=False, stop=(mi == nmm),
                                    skip_group_check=True)
                            for sg in range(SG):
                                for i in range(nt):
                                    mi += 1
                                    nc.tensor.matmul(
                                        agg[:, i, :],
                                        sel_t[:, nt * GPC + sg * nt + i, :],
                                        gbuf[:, nt * GPC + sg, :],
                                        start=False, stop=(mi == nmm),
                                        skip_group_check=True)
                            cols = slice(blk[0], blk[0] + nt)
                            if w == 0:
                                nc.vector.tensor_copy(agg_sb[:, cols, :],
                                                      agg[:, 0:nt, :])
                            else:
                                nc.vector.tensor_add(agg_sb[:, cols, :],
                                                     agg_sb[:, cols, :],
                                                     agg[:, 0:nt, :])
                            if w == cfg.NBLK - 1:
                                for t in blk:
                                    mlp_tile(l, t)
                    if l < L - 1:
                        emit_ag(half, (l + 1) % 2)

    nc.finalize()
    return nc


_CACHE = {}


def _get_program(cfg):
    key = (cfg.N, cfg.CAP, cfg.TPB, cfg.STRAY, cfg.NQ)
    if key not in _CACHE:
        _CACHE[key] = _build_program(cfg)
    return _CACHE[key]


def build_in_maps(cfg, inputs):
    params = _fold_params(
        cfg, inputs["x_emb"], inputs["etab"], inputs["w1"], inputs["b1"],
        inputs["w2"], inputs["b2"], inputs["gamma"], inputs["beta"],
        inputs["bn_mean"], inputs["bn_var"])
    per_core = _prepare_inputs(cfg, inputs["x"], inputs["edge_index"],
                               inputs["edge_attr"])
    in_maps = []
    for r in range(cfg.NCORE):
        m = dict(per_core[r])
        m.update({k: np.ascontiguousarray(v) for k, v in params.items()})
        in_maps.append(m)
    return in_maps


def kernel(**inputs) -> np.ndarray:
    cfg = CFG()
    in_maps = build_in_maps(cfg, inputs)   # sets cfg.STRAY from data
    nc = _get_program(cfg)
    res = run_bass_kernel_spmd(nc, in_maps, list(range(cfg.NCORE)))
    out = np.empty((cfg.N, cfg.D), np.float32)
    for r in range(cfg.NCORE):
        out[r * cfg.NOWN:(r + 1) * cfg.NOWN] = res.results[r]["out"][:cfg.NOWN]
    return out
